# revision 53
# baseline (speedup 1.0000x reference)
"""MLA (DeepSeek-style multi-head latent attention) distributed Bass kernel
for 8 TRN2 NeuronCores.

Problem shapes (hardcoded):
  x (2, 2048, 2048), DIM=2048, N_HEADS=16, Q_LORA=1536, KV_LORA=512,
  QK_NOPE=128, QK_ROPE=64, V_HEAD=128, causal SDPA, scale=192**-0.5.

Distribution / overlap strategy:
  phase 1 (token-parallel, 512 tokens/core): q_lora = x@wq_a.T and
    kv = x@wkv_a.T, shipped UNNORMALIZED together with per-token inv-rms rows
    so each AllGather can launch as soon as its slice of matmuls finishes
    (RMSNorm would otherwise couple all features and serialize compute->AG).
    Rope is applied to the shared k_pe here (per-token). Three AllGathers:
    kv (577 rows), q first half (768 rows), q second half (769 rows).
  phase 2 (head-parallel, 2 heads/core): k/v production (gated only on AG_kv)
    runs while the q AllGathers are in flight; then q production; then causal
    flash attention in the S^T formulation (kt on partitions, exp without
    max-subtraction - scores are provably small here). RMSNorm is applied at
    production time: row-broadcast multiplies for k/q (token axis = free) and
    per-partition activation scales for v (token axis = partitions).
  AllToAll per local head ships UNNORMALIZED attention outputs + softmax
    denominators (129-row shards); normalization happens in phase 3, keeping
    the attention inner loop free of serializing reductions.
  phase 3 (token-parallel): normalize per head, then out = attn@wo.T.

All matmul operands are bfloat16 (fp32 PSUM accumulation); fp32 is used for
the rms/softmax statistics chains. Activations are feature-major
[features(partitions), tokens(free)] so every matmul consumes operands
natively - there are no transposes anywhere in the kernel.
"""
import sys

sys.path.insert(0, "/opt/trn_rl_repo")

import numpy as np
import ml_dtypes

import concourse.bacc as bacc
import concourse.mybir as mybir
import concourse.tile as tile
from concourse.bass_utils import run_bass_kernel_spmd

BF = ml_dtypes.bfloat16
F32 = mybir.dt.float32
CD = mybir.dt.bfloat16
AFT = mybir.ActivationFunctionType

DIM = 2048
H = 16
QL = 1536
KVL = 512
DN = 128          # qk_nope
DR = 64           # qk_rope
DV = 128          # v head dim
B, S = 2, 2048
T = B * S
NC = 8
TPC = T // NC     # 512 tokens per core
HPC = H // NC     # 2 heads per core
SCALE = (DN + DR) ** -0.5
EPS = 1e-6
P = 128
CHUNK = 512
NCH = T // CHUNK
KVE = KVL + 2 * DR   # 640 phase-1 kv output cols (incl swapped-rope block)
KVG = KVL + DR + 2   # 578 gathered kv rows (+ kv inv_rms row + q inv_rms row)
QG1 = QL // 2        # 768
QG2 = QL // 2 + 1    # 769 (+ inv_rms row)


def build_nc():
    nc = bacc.Bacc("TRN2", target_bir_lowering=False, debug=False, num_devices=NC)

    xT = nc.declare_dram_parameter("xT", [DIM, TPC], CD, isOutput=False)
    xTf = nc.declare_dram_parameter("xTf", [DIM // P, P, T], CD, isOutput=False)
    wqaT = nc.declare_dram_parameter("wqaT", [QL // P, P, DIM], CD, isOutput=False)
    wkvaT = nc.declare_dram_parameter("wkvaT", [KVE // P, P, DIM], CD, isOutput=False)
    wqeT = nc.declare_dram_parameter("wqeT", [3, P, DIM], CD, isOutput=False)
    wkvbTk = nc.declare_dram_parameter("wkvbTk", [2, P, KVL], CD, isOutput=False)
    wkvbTv = nc.declare_dram_parameter("wkvbTv", [KVL, 2 * DV], CD, isOutput=False)
    woT = nc.declare_dram_parameter("woT", [DIM // P, P, DIM], CD, isOutput=False)
    cosb = nc.declare_dram_parameter("cosb", [P, S], CD, isOutput=False)
    sinb = nc.declare_dram_parameter("sinb", [P, S], CD, isOutput=False)
    cosc = nc.declare_dram_parameter("cosc", [P, TPC], CD, isOutput=False)
    sinc = nc.declare_dram_parameter("sinc", [P, TPC], CD, isOutput=False)
    perm = nc.declare_dram_parameter("perm", [P, P], CD, isOutput=False)
    dmaskP = nc.declare_dram_parameter("dmaskP", [P, 4 * CHUNK], CD, isOutput=False)
    outT = nc.declare_dram_parameter("outT", [DIM, TPC], CD, isOutput=True)

    def dma_split(dst_ap, src_ap, ncols, nsplit):
        """Issue a wide [128, ncols] DMA as nsplit column-chunks so they
        spread across DMA engines (~23 GB/s per engine)."""
        step = ncols // nsplit
        for i in range(nsplit):
            nc.sync.dma_start(dst_ap[:, i * step:(i + 1) * step],
                              src_ap[:, i * step:(i + 1) * step])

    dummy_in = nc.dram_tensor("dummy_in", [1, 16], CD)
    dummy_out = nc.dram_tensor("dummy_out", [NC, 16], CD, addr_space="Shared")
    agkv_in = nc.dram_tensor("agkv_in", [KVG, TPC], CD)
    agkv_out = nc.dram_tensor("agkv_out", [NC * KVG, TPC], CD, addr_space="Shared")
    a2a_in = [nc.dram_tensor(f"a2a_in{h}", [NC * DV, CHUNK], CD)
              for h in range(HPC)]
    a2a_out = [nc.dram_tensor(f"a2a_out{h}", [NC * DV, CHUNK], CD)
               for h in range(HPC)]
    groups = [list(range(NC))]

    with tile.TileContext(nc) as tc, \
         tc.tile_pool(name="const", bufs=1) as constp:
        # dummy first collective: absorbs the one-time all-rank rendezvous
        # BARRIER (and any cross-core launch skew) while phase-1 compute
        # runs, instead of paying it between phase 1 and the real AGs
        nc.gpsimd.collective_compute(
            "AllGather", mybir.AluOpType.bypass, replica_groups=groups,
            ins=[dummy_in.ap().opt()], outs=[dummy_out.ap().opt()])
        ones_f = constp.tile([P, 1], F32, tag="onesf")
        nc.any.memset(ones_f[:], 1.0)
        ones_col = constp.tile([P, 1], CD, tag="onesc")
        nc.vector.tensor_copy(ones_col[:], ones_f[:])
        onesr_f = constp.tile([1, P], F32, tag="onesrf")
        nc.any.memset(onesr_f[:], 1.0)
        ones_row = constp.tile([1, P], CD, tag="onesr")
        nc.vector.tensor_copy(ones_row[:], onesr_f[:])
        one_f11 = constp.tile([1, 1], F32, tag="onef11")
        nc.any.memset(one_f11[:], 1.0)
        one_1x1 = constp.tile([1, 1], CD, tag="one11")
        nc.vector.tensor_copy(one_1x1[:], one_f11[:])
        eps_t = constp.tile([1, 1], F32, tag="eps")
        nc.any.memset(eps_t[:], EPS)
        # touch Exp once now so the ACT table load happens during phase 1,
        # not at the first attention softmax
        warm_e = constp.tile([1, 1], F32, tag="warme")
        nc.scalar.activation(warm_e[:], eps_t[:], AFT.Exp, bias=0.0, scale=1.0)

        _wp3cm = tc.tile_pool(name="p3w", bufs=1)
        wp3 = _wp3cm.__enter__()
        wo_sb = [None] * (DIM // P)

        # persistent phase-2 tensors; zero-fills go first on the gpsimd queue
        _qp2cm = tc.tile_pool(name="p2q", bufs=1)
        qp2 = _qp2cm.__enter__()
        _kp2cm = tc.tile_pool(name="p2k", bufs=1)
        kp2 = _kp2cm.__enter__()
        _vp2cm = tc.tile_pool(name="p2v", bufs=T // P)
        vp2 = _vp2cm.__enter__()
        q_n = [qp2.tile([P, T], CD, tag=f"qn{h}", name=f"qn{h}") for h in range(HPC)]
        q_rope = [qp2.tile([P, T], CD, tag=f"qrope{h}", name=f"qrope{h}")
                  for h in range(HPC)]
        k_n = [kp2.tile([P, T], CD, tag=f"kn{h}", name=f"kn{h}") for h in range(HPC)]
        k_pe = kp2.tile([2 * DR, T], CD, tag="kpe")
        v_tok = [vp2.tile([P, HPC * DV], CD, tag="vtok", name=f"vtok{i}")
                 for i in range(T // P)]
        nc.any.memset(k_pe[DR:2 * DR, :], 0.0)
        for h in range(HPC):
            nc.any.memset(q_rope[h][DR:2 * DR, :], 0.0)

        # fused q weight loads NOW (t~0) so the fused-q chunks can start the
        # instant phase 1 finishes (SBUF for this is freed below by keeping
        # the phase-1 square tiles in a ring instead of 16 persistent tiles)
        _wqecm = tc.tile_pool(name="p2wqe", bufs=1)
        wqep = _wqecm.__enter__()
        wqe_sb = []
        for m in range(3):
            w = wqep.tile([P, DIM], CD, tag=f"wqe{m}", name=f"wqe{m}")
            nc.sync.dma_start(w[:], wqeT[m, :, :])
            wqe_sb.append(w)

        # ---------------- phase 1 ------------------------------------------
        with tc.tile_pool(name="p1x", bufs=1) as xp, \
             tc.tile_pool(name="p1w", bufs=3) as wp1, \
             tc.tile_pool(name="p1ps", bufs=3, space="PSUM") as ps1, \
             tc.tile_pool(name="p1ssps", bufs=2, space="PSUM") as ssps1, \
             tc.tile_pool(name="p1sq", bufs=4) as sqp, \
             tc.tile_pool(name="p1misc", bufs=2) as mp1, \
             tc.tile_pool(name="p1out", bufs=3) as op1:

            # first kv weight tile goes out ahead of the x tiles so the
            # first matmul isn't stuck behind 2MB of x DMA
            wkv0 = wp1.tile([P, DIM], CD, tag="w1")
            nc.sync.dma_start(wkv0[:], wkvaT[0, :, :])

            xts = []
            for kb in range(DIM // P):
                xt = xp.tile([P, TPC], CD, tag=f"x{kb}")
                nc.sync.dma_start(xt[:], xT[kb * P:(kb + 1) * P, :])
                xts.append(xt)

            cosc_sb = mp1.tile([P, TPC], CD, tag="cosc")
            nc.sync.dma_start(cosc_sb[:], cosc[:, :])
            sinc_sb = mp1.tile([P, TPC], CD, tag="sinc")
            nc.sync.dma_start(sinc_sb[:], sinc[:, :])

            def lora_proj(w_param, nm, out_cb, w0=None):
                for m in range(nm):
                    if m == 0 and w0 is not None:
                        wt = w0
                    else:
                        wt = wp1.tile([P, DIM], CD, tag="w1")
                        nc.sync.dma_start(wt[:], w_param[m, :, :])
                    ps = ps1.tile([P, TPC], F32, tag="ps1")
                    for kb in range(DIM // P):
                        nc.tensor.matmul(ps[:], wt[:, kb * P:(kb + 1) * P], xts[kb][:],
                                         start=(kb == 0), stop=(kb == DIM // P - 1))
                    out_cb(m, ps)

            def invrms_row(ss, nfeat, dst, dst_row):
                """accumulated sumsq -> 1/rms row (CD) -> DMA to dst[dst_row]."""
                srt = mp1.tile([1, TPC], F32, tag="srt")
                nc.scalar.activation(srt[:], ss[:], AFT.Sqrt,
                                     bias=eps_t[:], scale=1.0 / nfeat)
                inv_f = mp1.tile([1, TPC], F32, tag="invf")
                nc.vector.reciprocal_approx_fast(inv_f[:], srt[:])
                inv = mp1.tile([1, TPC], CD, tag="inv")
                nc.vector.tensor_copy(inv[:], inv_f[:])
                nc.scalar.dma_start(dst[dst_row:dst_row + 1, :], inv[:])

            # sumsq accumulated INLINE (ss matmul per m-tile) so the square
            # tiles live in a 4-deep ring instead of 16 persistent tiles --
            # that SBUF headroom is what pays for the early wqe load above
            ss_kv = ssps1.tile([1, TPC], F32, tag="sskv")
            ss_q = ssps1.tile([1, TPC], F32, tag="ssq")

            # --- kv branch (payload shipped unnormalized + inv-rms row)
            def kv_cb(m, ps):
                if m < 4:
                    kv = op1.tile([P, TPC], CD, tag="pay")
                    nc.scalar.copy(kv[:], ps[:])
                    nc.scalar.dma_start(agkv_in[m * P:(m + 1) * P, :], kv[:])
                    sq = sqp.tile([P, TPC], CD, tag="sq")
                    nc.scalar.square(sq[:], ps[:])
                    nc.tensor.matmul(ss_kv[:], ones_col[:], sq[:],
                                     start=(m == 0), stop=(m == 3))
                else:
                    # rows 0:64 deinterleaved kpe [r;i], rows 64:128 [i;r]
                    t_a = op1.tile([DR, TPC], CD, tag="ropea")
                    nc.vector.tensor_mul(t_a[:], ps[0:DR, :], cosc_sb[0:DR, :])
                    t_b = op1.tile([DR, TPC], CD, tag="ropeb")
                    nc.vector.tensor_mul(t_b[:], ps[DR:2 * DR, :], sinc_sb[0:DR, :])
                    kpe = op1.tile([DR, TPC], CD, tag="ropeo")
                    nc.vector.tensor_add(kpe[:], t_a[:], t_b[:])
                    nc.scalar.dma_start(agkv_in[KVL:KVL + DR, :], kpe[:])

            lora_proj(wkvaT, KVE // P, kv_cb, w0=wkv0)
            invrms_row(ss_kv, KVL, agkv_in, KVL + DR)

            # --- q branch: only the rms statistic is needed token-parallel;
            # it rides the kv AllGather as one extra row. The q heads are
            # produced by the fused W_eff = wq_a.T @ wq_b.T weight over ALL
            # tokens in phase 2 (no q AllGather at all).
            def q_cb(m, ps):
                sq = sqp.tile([P, TPC], CD, tag="sq")
                nc.scalar.square(sq[:], ps[:])
                nc.tensor.matmul(ss_q[:], ones_col[:], sq[:],
                                 start=(m == 0), stop=(m == QL // P - 1))

            lora_proj(wqaT, QL // P, q_cb)
            invrms_row(ss_q, QL, agkv_in, KVL + DR + 1)

            nc.gpsimd.collective_compute(
                "AllGather", mybir.AluOpType.bypass, replica_groups=groups,
                ins=[agkv_in.ap().opt()], outs=[agkv_out.ap().opt()])

        # ---------------- phase 2 ------------------------------------------
        if True:
            with tc.tile_pool(name="p2cs", bufs=2) as csp, \
                 tc.tile_pool(name="p2w", bufs=1) as wp2, \
                 tc.tile_pool(name="p2actq", bufs=12) as actp, \
                 tc.tile_pool(name="p2actkv", bufs=6) as actkvp, \
                 tc.tile_pool(name="p2misc", bufs=3) as mp2, \
                 tc.tile_pool(name="p2nrm", bufs=3) as nrmp:
                perm_sb = mp2.tile([P, P], CD, tag="perm")
                nc.sync.dma_start(perm_sb[:], perm[:, :])
                wkvk_sb = []
                for m in range(2):
                    w = wp2.tile([P, KVL], CD, tag=f"wkvk{m}", name=f"wkvk{m}")
                    nc.sync.dma_start(w[:], wkvbTk[m, :, :])
                    wkvk_sb.append(w)
                wkvv_sb = []
                for kb in range(KVL // P):
                    w = wp2.tile([P, 2 * DV], CD, tag=f"wkvv{kb}", name=f"wkvv{kb}")
                    nc.sync.dma_start(w[:], wkvbTv[kb * P:(kb + 1) * P, :])
                    wkvv_sb.append(w)

                # --- fused q production: q heads for ALL tokens from local
                # x (replicated input) and the host-fused W_eff weight; only
                # the tiny inv-rms AG is a cross-core dependency, so chunks
                # 0..2 fill the tensor-idle window while AG_kv is in flight
                def q_chunk(qc, psq, psqb):
                    c0 = qc * CHUNK
                    iq = nrmp.tile([1, CHUNK], CD, tag="iq")
                    nc.sync.dma_start(
                        iq[:],
                        agkv_out[qc * KVG + KVL + DR + 1:
                                 qc * KVG + KVL + DR + 2, :])
                    bq_ps = psqb.tile([P, CHUNK], F32, tag="bq")
                    nc.tensor.matmul(bq_ps[:], ones_row[:], iq[:],
                                     start=True, stop=True)
                    bq = nrmp.tile([P, CHUNK], CD, tag="bqsb")
                    nc.scalar.copy(bq[:], bq_ps[:])
                    pss = [psq.tile([P, CHUNK], F32, tag=f"qps{m}", name=f"qps{m}")
                           for m in range(3)]
                    for kb in range(DIM // P):
                        at = actp.tile([P, CHUNK], CD, tag="actq")
                        nc.sync.dma_start(at[:], xTf[kb, :, c0:c0 + CHUNK])
                        for m in range(3):
                            nc.tensor.matmul(pss[m][:],
                                             wqe_sb[m][:, kb * P:(kb + 1) * P],
                                             at[:],
                                             start=(kb == 0),
                                             stop=(kb == DIM // P - 1))
                    for h in range(HPC):
                        nc.vector.tensor_mul(q_n[h][:, c0:c0 + CHUNK],
                                             pss[h][:], bq[:])
                    qr_raw = mp2.tile([P, CHUNK], CD, tag="qrraw")
                    nc.vector.tensor_mul(qr_raw[:], pss[2][:], bq[:])
                    ps_sw = psq.tile([P, CHUNK], F32, tag="qps0")
                    nc.tensor.matmul(ps_sw[:], perm_sb[:], qr_raw[:],
                                     start=True, stop=True)
                    pcol = c0 % S
                    cos_q = csp.tile([P, CHUNK], CD, tag="cosq")
                    nc.sync.dma_start(cos_q[:], cosb[:, pcol:pcol + CHUNK])
                    sin_q = csp.tile([P, CHUNK], CD, tag="sinq")
                    nc.sync.dma_start(sin_q[:], sinb[:, pcol:pcol + CHUNK])
                    t_a = mp2.tile([P, CHUNK], CD, tag="qra")
                    nc.vector.tensor_mul(t_a[:], qr_raw[:], cos_q[:])
                    t_b = mp2.tile([P, CHUNK], CD, tag="qrb")
                    nc.vector.tensor_mul(t_b[:], ps_sw[:], sin_q[:])
                    qr_fin = mp2.tile([P, CHUNK], CD, tag="qrfin")
                    nc.vector.tensor_add(qr_fin[:], t_a[:], t_b[:])
                    for h in range(HPC):
                        nc.sync.dma_start(q_rope[h][0:DR, c0:c0 + CHUNK],
                                          qr_fin[h * DR:(h + 1) * DR, :])

                qscope = tc.tile_pool(name="p2psq", bufs=2, space="PSUM")
                psq = qscope.__enter__()
                qbscope = tc.tile_pool(name="p2psqb", bufs=2, space="PSUM")
                psqb = qbscope.__enter__()
                for qc in range(5):
                    q_chunk(qc, psq, psqb)
                qbscope.__exit__(None, None, None)
                qscope.__exit__(None, None, None)

                # --- k/v production (gated only on AG_kv)
                kvscope = tc.tile_pool(name="p2psk", bufs=1, space="PSUM")
                psk = kvscope.__enter__()
                vscope = tc.tile_pool(name="p2psv", bufs=3, space="PSUM")
                psv = vscope.__enter__()
                bscope = tc.tile_pool(name="p2psb", bufs=1, space="PSUM")
                psb = bscope.__enter__()
                for qc in range(NCH):
                    c0 = qc * CHUNK
                    actkv_ts = []
                    for kb in range(KVL // P):
                        at = actkvp.tile([P, CHUNK], CD, tag="actkv")
                        nc.sync.dma_start(
                            at[:],
                            agkv_out[qc * KVG + kb * P: qc * KVG + (kb + 1) * P, :])
                        actkv_ts.append(at)
                    ikv = nrmp.tile([1, CHUNK], CD, tag="ikv")
                    nc.sync.dma_start(
                        ikv[:],
                        agkv_out[qc * KVG + KVL + DR: qc * KVG + KVL + DR + 1, :])
                    bkv_ps = psb.tile([P, CHUNK], F32, tag="bkv")
                    nc.tensor.matmul(bkv_ps[:], ones_row[:], ikv[:],
                                     start=True, stop=True)
                    bkv = nrmp.tile([P, CHUNK], CD, tag="bkvsb")
                    nc.scalar.copy(bkv[:], bkv_ps[:])
                    kps = [psk.tile([P, CHUNK], F32, tag=f"kps{m}", name=f"kps{m}")
                           for m in range(2)]
                    for kb in range(KVL // P):
                        for m in range(2):
                            nc.tensor.matmul(kps[m][:],
                                             wkvk_sb[m][:, kb * P:(kb + 1) * P],
                                             actkv_ts[kb][:],
                                             start=(kb == 0), stop=(kb == KVL // P - 1))
                    for h in range(HPC):
                        nc.vector.tensor_mul(k_n[h][:, c0:c0 + CHUNK], kps[h][:], bkv[:])
                    nc.sync.dma_start(
                        k_pe[0:DR, c0:c0 + CHUNK],
                        agkv_out[qc * KVG + KVL: qc * KVG + KVL + DR, :])
                    for ktc in range(CHUNK // P):
                        # inv_kv as a [128,1] column for this kt tile (K=1 matmul)
                        col_ps = psb.tile([P, 1], F32, tag="colp")
                        nc.tensor.matmul(col_ps[:],
                                         ikv[0:1, ktc * P:(ktc + 1) * P],
                                         one_1x1[:], start=True, stop=True)
                        vcol = nrmp.tile([P, 1], F32, tag="vcol")
                        nc.scalar.copy(vcol[:], col_ps[:])
                        vps = psv.tile([P, HPC * DV], F32, tag="vps")
                        for kb in range(KVL // P):
                            nc.tensor.matmul(vps[:],
                                             actkv_ts[kb][:, ktc * P:(ktc + 1) * P],
                                             wkvv_sb[kb][:],
                                             start=(kb == 0), stop=(kb == KVL // P - 1))
                        nc.scalar.activation(v_tok[qc * (CHUNK // P) + ktc][:], vps[:],
                                             AFT.Copy, bias=0.0, scale=vcol[:])
                bscope.__exit__(None, None, None)
                vscope.__exit__(None, None, None)
                kvscope.__exit__(None, None, None)

                # --- remaining fused-q chunks
                qscope = tc.tile_pool(name="p2psq2", bufs=2, space="PSUM")
                psq = qscope.__enter__()
                qbscope = tc.tile_pool(name="p2psqb2", bufs=2, space="PSUM")
                psqb = qbscope.__enter__()
                for qc in range(5, NCH):
                    q_chunk(qc, psq, psqb)
                qbscope.__exit__(None, None, None)
                qscope.__exit__(None, None, None)
            _wqecm.__exit__(None, None, None)

            # prefetch half the phase-3 wo weights while attention runs
            for m in range(DIM // P // 2):
                wt = wp3.tile([P, DIM], CD, tag=f"wo{m}", name=f"wo{m}")
                nc.sync.dma_start(wt[:], woT[m, :, :])
                wo_sb[m] = wt

            # ----- causal attention (S^T formulation), head-outer for A2A overlap
            # kt-pairs share one [128,1024] PSUM score tile so exp runs as a
            # single wide ACTIVATE (halves the per-instruction overhead); the
            # causal diagonal is masked by a DVE multiply with precomputed 0/1
            # masks (keeps gpsimd off the critical path); outputs are
            # normalized here (den broadcast + reciprocal) so the A2A ships
            # only DV rows and phase 3 is a pure matmul
            with tc.tile_pool(name="apt", bufs=4) as ptp, \
                 tc.tile_pool(name="amask", bufs=1) as mkp, \
                 tc.tile_pool(name="aout", bufs=2, space="PSUM") as outp, \
                 tc.tile_pool(name="aden", bufs=1, space="PSUM") as denp, \
                 tc.tile_pool(name="abc", bufs=1, space="PSUM") as bcp, \
                 tc.tile_pool(name="ast", bufs=2, space="PSUM") as stp, \
                 tc.tile_pool(name="afin", bufs=3) as finp:
                dmask_sb = mkp.tile([P, 4 * CHUNK], CD, tag="dmask")
                nc.sync.dma_start(dmask_sb[:], dmaskP[:, :])
                for h in range(HPC):
                    for b in range(B):
                        for qcl in range(S // CHUNK):
                            qg = b * (S // CHUNK) + qcl
                            q0 = qg * CHUNK
                            nkt = (CHUNK // P) * (qcl + 1)
                            npair = nkt // 2
                            out_ps = outp.tile([P, CHUNK], F32, tag="out")
                            den_ps = denp.tile([1, CHUNK], F32, tag="den")

                            def qk_pair(pi):
                                st_ps = stp.tile([P, 2 * CHUNK], F32, tag="st",
                                                 name=f"st{pi}")
                                for s_ in range(2):
                                    kt = 2 * pi + s_
                                    kcol = b * S + kt * P
                                    sl = st_ps[:, s_ * CHUNK:(s_ + 1) * CHUNK]
                                    nc.tensor.matmul(sl,
                                                     k_n[h][:, kcol:kcol + P],
                                                     q_n[h][:, q0:q0 + CHUNK],
                                                     start=True, stop=False)
                                for s_ in range(2):
                                    kt = 2 * pi + s_
                                    kcol = b * S + kt * P
                                    nc.tensor.matmul(
                                        st_ps[:, s_ * CHUNK:(s_ + 1) * CHUNK],
                                        k_pe[:, kcol:kcol + P],
                                        q_rope[h][:, q0:q0 + CHUNK],
                                        start=False, stop=True)
                                pt = ptp.tile([P, 2 * CHUNK], CD, tag="pt",
                                              name=f"pt{pi}")
                                nc.scalar.activation(pt[:], st_ps[:], AFT.Exp,
                                                     bias=0.0, scale=SCALE)
                                halves = []
                                for s_ in range(2):
                                    kt = 2 * pi + s_
                                    r = kt - (CHUNK // P) * qcl
                                    if r >= 0:  # diagonal block: mask j >= p+128r
                                        ptm = ptp.tile([P, CHUNK], CD, tag="ptm",
                                                       name=f"ptm{pi}_{s_}")
                                        nc.vector.tensor_mul(
                                            ptm[:],
                                            pt[:, s_ * CHUNK:(s_ + 1) * CHUNK],
                                            dmask_sb[:, r * CHUNK:(r + 1) * CHUNK])
                                        halves.append(ptm[:])
                                    else:
                                        halves.append(
                                            pt[:, s_ * CHUNK:(s_ + 1) * CHUNK])
                                return halves

                            pend = {}

                            def av_pair(pi, halves):
                                for s_ in range(2):
                                    kt = 2 * pi + s_
                                    nc.tensor.matmul(
                                        out_ps[:],
                                        v_tok[(b * S // P) + kt][:,
                                                                 h * DV:(h + 1) * DV],
                                        halves[s_],
                                        start=(kt == 0), stop=(kt == nkt - 1))
                                pd = ptp.tile([P, CHUNK], CD, tag="pden",
                                              name=f"pden{pi}")
                                nc.vector.tensor_add(pd[:], halves[0],
                                                     halves[1])
                                if pi % 2 == 0:
                                    pend[0] = pd
                                else:
                                    pd4 = ptp.tile([P, CHUNK], CD, tag="pden4",
                                                   name=f"pden4_{pi}")
                                    nc.vector.tensor_add(pd4[:], pend[0][:], pd[:])
                                    nc.tensor.matmul(den_ps[:], ones_col[:], pd4[:],
                                                     start=(pi == 1),
                                                     stop=(pi == npair - 1))

                            prev = None
                            for pi in range(npair):
                                cur = qk_pair(pi)
                                if prev is not None:
                                    av_pair(pi - 1, prev)
                                prev = cur
                            av_pair(npair - 1, prev)
                            # normalize here: broadcast den to 128 rows,
                            # reciprocal, scale the output as it leaves PSUM
                            den_c = finp.tile([1, CHUNK], CD, tag="denc")
                            nc.scalar.copy(den_c[:], den_ps[:])
                            bc_ps = bcp.tile([P, CHUNK], F32, tag="bc")
                            nc.tensor.matmul(bc_ps[:], ones_row[:], den_c[:],
                                             start=True, stop=True)
                            binv = finp.tile([P, CHUNK], F32, tag="binv")
                            nc.vector.reciprocal_approx_fast(binv[:], bc_ps[:])
                            attn = finp.tile([P, CHUNK], CD, tag="attn")
                            nc.vector.tensor_mul(attn[:], out_ps[:], binv[:])
                            nc.sync.dma_start(
                                a2a_in[h][qg * DV:(qg + 1) * DV, :], attn[:])
                    nc.gpsimd.collective_compute(
                        "AllToAll", mybir.AluOpType.bypass, replica_groups=groups,
                        ins=[a2a_in[h].ap().opt()], outs=[a2a_out[h].ap().opt()])

        _vp2cm.__exit__(None, None, None)
        _kp2cm.__exit__(None, None, None)
        _qp2cm.__exit__(None, None, None)

        # ---------------- phase 3 ------------------------------------------
        with tc.tile_pool(name="p3r", bufs=1) as rp3, \
             tc.tile_pool(name="p3ps", bufs=2, space="PSUM") as ps3, \
             tc.tile_pool(name="p3o", bufs=3) as op3:
            # heads arriving via the first A2A load first; the woT weight
            # loads sit between the two gated groups so they prefetch
            # before the second A2A lands
            rhs_t = [None] * H
            order = [g for g in range(H) if g % HPC == 0]
            order2 = [g for g in range(H) if g % HPC != 0]

            def load_head(g):
                blk = (g // HPC) * DV
                rt = rp3.tile([P, CHUNK], CD, tag=f"r{g}", name=f"r{g}")
                nc.sync.dma_start(rt[:], a2a_out[g % HPC][blk:blk + DV, :])
                rhs_t[g] = rt

            for m in range(DIM // P // 2, DIM // P):
                wt = wp3.tile([P, DIM], CD, tag=f"wo{m}", name=f"wo{m}")
                nc.sync.dma_start(wt[:], woT[m, :, :])
                wo_sb[m] = wt
            for g in order:
                load_head(g)
            for g in order2:
                load_head(g)

            partials = []
            for m in range(DIM // P):
                ps = ps3.tile([P, CHUNK], F32, tag="ps3")
                for i, g in enumerate(order):
                    nc.tensor.matmul(ps[:], wo_sb[m][:, g * P:(g + 1) * P], rhs_t[g][:],
                                     start=(i == 0), stop=(i == len(order) - 1))
                pa = rp3.tile([P, CHUNK], F32, tag=f"pa{m}", name=f"pa{m}")
                nc.scalar.copy(pa[:], ps[:])
                partials.append(pa)
            for m in range(DIM // P):
                ps = ps3.tile([P, CHUNK], F32, tag="ps3")
                for i, g in enumerate(order2):
                    nc.tensor.matmul(ps[:], wo_sb[m][:, g * P:(g + 1) * P], rhs_t[g][:],
                                     start=(i == 0), stop=(i == len(order2) - 1))
                ot = op3.tile([P, CHUNK], CD, tag="ot")
                nc.vector.tensor_add(ot[:], ps[:], partials[m][:])
                nc.sync.dma_start(outT[m * P:(m + 1) * P, :], ot[:])
        _wp3cm.__exit__(None, None, None)

    nc.compile()
    return nc


def _tile_kxm(w, nk, nm):
    """(nk*128, nm*128) -> (nm, 128, nk*128): [m][p][kt*128+j] = w[kt*128+p, m*128+j]."""
    return np.ascontiguousarray(
        w.reshape(nk, P, nm, P).transpose(2, 1, 0, 3).reshape(nm, P, nk * P))


_CACHE = {}


def _prep(inputs):
    x = np.asarray(inputs["x"], np.float32)
    fc = np.asarray(inputs["freqs_cos"], np.float32)
    fs = np.asarray(inputs["freqs_sin"], np.float32)
    wq_a = np.asarray(inputs["wq_a"], np.float32)
    q_norm_w = np.asarray(inputs["q_norm_w"], np.float32)
    wq_b = np.asarray(inputs["wq_b"], np.float32)
    wkv_a = np.asarray(inputs["wkv_a"], np.float32)
    kv_norm_w = np.asarray(inputs["kv_norm_w"], np.float32)
    wkv_b = np.asarray(inputs["wkv_b"], np.float32)
    wo = np.asarray(inputs["wo"], np.float32)

    x_flat = x.reshape(T, DIM)
    xTf_t = np.ascontiguousarray(x_flat.T).reshape(DIM // P, P, T).astype(BF)

    wqaT_t = _tile_kxm(wq_a.T, DIM // P, QL // P)

    at = wkv_a.T                                     # (DIM, 576)
    Rw = at[:, KVL::2]
    Iw = at[:, KVL + 1::2]
    wkvaT_t = _tile_kxm(np.concatenate([at[:, :KVL], Rw, Iw, Iw, Rw], axis=1),
                        DIM // P, KVE // P)

    wqb_sT = (wq_b * q_norm_w[None, :]).T            # (QL, H*192)
    wkvb_sT = (wkv_b * kv_norm_w[None, :]).T         # (KVL, H*256)

    woT_t = _tile_kxm(wo.T, DIM // P, DIM // P)

    cT, sT = fc.T, fs.T
    cosbM = np.concatenate([cT, cT, cT, cT], axis=0)
    sinbM = np.concatenate([-sT, sT, -sT, sT], axis=0)
    permM = np.zeros((P, P), np.float32)
    permM[np.arange(P) ^ 32, np.arange(P)] = 1.0

    # causal diagonal masks: mask_r[p, j] = (j >= p + 128*r) for the 4
    # possible k-tile offsets within a 512-token q chunk
    jj = np.arange(CHUNK)[None, :]
    pp = np.arange(P)[:, None]
    dmaskM = np.concatenate(
        [(jj >= pp + P * r).astype(np.float32) for r in range(4)], axis=1)

    in_maps = []
    for c in range(NC):
        h0, h1 = 2 * c, 2 * c + 1
        qb = [wqb_sT[:, h * 192: h * 192 + DN] for h in (h0, h1)]
        for h in (h0, h1):
            rope = wqb_sT[:, h * 192 + DN:(h + 1) * 192]
            qb.append(rope[:, 0::2])
            qb.append(rope[:, 1::2])
        # fold wq_a through: q^h = inv_rms ⊙ (x @ (wq_a.T @ wqb_sT^h))
        wqe_c = wq_a.T @ np.concatenate(qb, axis=1)      # (DIM, 384)
        wqeT_ct = _tile_kxm(wqe_c, DIM // P, 3)

        kn = [wkvb_sT[:, h * 256: h * 256 + DN] for h in (h0, h1)]
        vv = [wkvb_sT[:, h * 256 + DN: (h + 1) * 256] for h in (h0, h1)]
        wkvbTk_c = _tile_kxm(np.concatenate(kn, axis=1), KVL // P, 2)
        wkvbTv_c = np.ascontiguousarray(np.concatenate(vv, axis=1))

        pos0 = (c * TPC) % S
        in_maps.append({
            "xT": np.ascontiguousarray(x_flat[c * TPC:(c + 1) * TPC].T).astype(BF),
            "wqaT": wqaT_t.astype(BF), "wkvaT": wkvaT_t.astype(BF),
            "wqeT": wqeT_ct.astype(BF), "xTf": xTf_t,
            "wkvbTk": wkvbTk_c.astype(BF),
            "wkvbTv": wkvbTv_c.astype(BF),
            "woT": woT_t.astype(BF), "cosb": cosbM.astype(BF),
            "sinb": sinbM.astype(BF), "perm": permM.astype(BF),
            "dmaskP": dmaskM.astype(BF),
            "cosc": np.ascontiguousarray(cosbM[:, pos0:pos0 + TPC]).astype(BF),
            "sinc": np.ascontiguousarray(sinbM[:, pos0:pos0 + TPC]).astype(BF),
        })
    return in_maps


def kernel(**inputs):
    in_maps = _prep(inputs)
    if "nc" not in _CACHE:
        _CACHE["nc"] = build_nc()
    r = run_bass_kernel_spmd(_CACHE["nc"], in_maps, list(range(NC)))
    out_flat = np.empty((T, DIM), np.float32)
    for c in range(NC):
        out_flat[c * TPC:(c + 1) * TPC] = r.results[c]["outT"].T.astype(np.float32)
    return out_flat.reshape(B, S, DIM)



# revision 55
# speedup vs baseline: 1.0405x; 1.0405x over previous
"""MLA (DeepSeek-style multi-head latent attention) distributed Bass kernel
for 8 TRN2 NeuronCores.

Problem shapes (hardcoded):
  x (2, 2048, 2048), DIM=2048, N_HEADS=16, Q_LORA=1536, KV_LORA=512,
  QK_NOPE=128, QK_ROPE=64, V_HEAD=128, causal SDPA, scale=192**-0.5.

Distribution / overlap strategy:
  A dummy 16-element AllGather is the kernel's first instruction: the
    one-time all-rank rendezvous BARRIER (30-200us, the dominant source of
    run-to-run variance) attaches to it and overlaps phase-1 compute.
  phase 1 (token-parallel, 512 tokens/core): kv = x@wkv_a.T shipped
    UNNORMALIZED, plus per-token inv-rms rows for BOTH branches (sum-of-
    squares accumulated inline so square tiles live in a 4-deep ring).
    Rope is applied to the shared k_pe here. ONE AllGather: 578 rows =
    kv_lora(512) + k_pe(64) + kv inv-rms + q inv-rms.
  q path is FUSED: q^h = inv_rms(q_lora) * (x @ [wq_a.T @ (wq_b*q_norm).T]),
    with the weight product folded on the host per core's 2 heads. Each core
    streams the REPLICATED x (xTf) and produces q for its heads over ALL
    tokens locally - no q AllGather. Chunks 0-2 are issued before k/v
    production so they fill the tensor-idle window while AG_kv is in flight
    (the fused weight is DMA'd at t~0 for the same reason).
  phase 2 (head-parallel, 2 heads/core): k/v production (gated only on
    AG_kv), then causal flash attention in the S^T formulation (kt on
    partitions, exp without max-subtraction - scores are provably small).
    exp runs one [128,1024] ACTIVATE per kt-pair (2 PSUM banks); the causal
    diagonal is masked by DVE multiplies with precomputed 0/1 masks.
    Attention outputs are normalized in place (denominator broadcast via a
    K=1 matmul + reciprocal) so each AllToAll ships plain 128-row shards.
  phase 3 (token-parallel): out = attn@wo.T, two head-groups so the first
    half starts as soon as the first AllToAll lands; bf16 output.

All matmul operands are bfloat16 (fp32 PSUM accumulation); fp32 is used for
the rms/softmax statistics chains. Activations are feature-major
[features(partitions), tokens(free)] so every matmul consumes operands
natively - there are no transposes anywhere in the kernel.
"""
import sys

sys.path.insert(0, "/opt/trn_rl_repo")

import numpy as np
import ml_dtypes

import concourse.bacc as bacc
import concourse.mybir as mybir
import concourse.tile as tile
from concourse.bass_utils import run_bass_kernel_spmd

BF = ml_dtypes.bfloat16
F32 = mybir.dt.float32
CD = mybir.dt.bfloat16
AFT = mybir.ActivationFunctionType

DIM = 2048
H = 16
QL = 1536
KVL = 512
DN = 128          # qk_nope
DR = 64           # qk_rope
DV = 128          # v head dim
B, S = 2, 2048
T = B * S
NC = 8
TPC = T // NC     # 512 tokens per core
HPC = H // NC     # 2 heads per core
SCALE = (DN + DR) ** -0.5
EPS = 1e-6
P = 128
CHUNK = 512
NCH = T // CHUNK
KVE = KVL + 2 * DR   # 640 phase-1 kv output cols (incl swapped-rope block)
KVG = KVL + DR + 2   # 578 gathered kv rows (+ kv inv_rms row + q inv_rms row)
QG1 = QL // 2        # 768
QG2 = QL // 2 + 1    # 769 (+ inv_rms row)


def build_nc():
    nc = bacc.Bacc("TRN2", target_bir_lowering=False, debug=False, num_devices=NC)

    xT = nc.declare_dram_parameter("xT", [DIM, TPC], CD, isOutput=False)
    xTf = nc.declare_dram_parameter("xTf", [DIM // P, P, T], CD, isOutput=False)
    wqaT = nc.declare_dram_parameter("wqaT", [QL // P, P, DIM], CD, isOutput=False)
    wkvaT = nc.declare_dram_parameter("wkvaT", [KVE // P, P, DIM], CD, isOutput=False)
    wqeT = nc.declare_dram_parameter("wqeT", [3, P, DIM], CD, isOutput=False)
    wkvbTk = nc.declare_dram_parameter("wkvbTk", [2, P, KVL], CD, isOutput=False)
    wkvbTv = nc.declare_dram_parameter("wkvbTv", [KVL, 2 * DV], CD, isOutput=False)
    woT = nc.declare_dram_parameter("woT", [DIM // P, P, DIM], CD, isOutput=False)
    cosb = nc.declare_dram_parameter("cosb", [P, S], CD, isOutput=False)
    sinb = nc.declare_dram_parameter("sinb", [P, S], CD, isOutput=False)
    cosc = nc.declare_dram_parameter("cosc", [P, TPC], CD, isOutput=False)
    sinc = nc.declare_dram_parameter("sinc", [P, TPC], CD, isOutput=False)
    perm = nc.declare_dram_parameter("perm", [P, P], CD, isOutput=False)
    dmaskP = nc.declare_dram_parameter("dmaskP", [P, 4 * CHUNK], CD, isOutput=False)
    outT = nc.declare_dram_parameter("outT", [DIM, TPC], CD, isOutput=True)

    def dma_split(dst_ap, src_ap, ncols, nsplit):
        """Issue a wide [128, ncols] DMA as nsplit column-chunks so they
        spread across DMA engines (~23 GB/s per engine)."""
        step = ncols // nsplit
        for i in range(nsplit):
            nc.sync.dma_start(dst_ap[:, i * step:(i + 1) * step],
                              src_ap[:, i * step:(i + 1) * step])

    dummy_in = nc.dram_tensor("dummy_in", [1, 16], CD)
    dummy_out = nc.dram_tensor("dummy_out", [NC, 16], CD, addr_space="Shared")
    agkv_in = nc.dram_tensor("agkv_in", [KVG, TPC], CD)
    agkv_out = nc.dram_tensor("agkv_out", [NC * KVG, TPC], CD, addr_space="Shared")
    a2a_in = [nc.dram_tensor(f"a2a_in{h}", [NC * DV, CHUNK], CD)
              for h in range(HPC)]
    a2a_out = [nc.dram_tensor(f"a2a_out{h}", [NC * DV, CHUNK], CD)
               for h in range(HPC)]
    groups = [list(range(NC))]

    with tile.TileContext(nc) as tc, \
         tc.tile_pool(name="const", bufs=1) as constp:
        # dummy first collective: absorbs the one-time all-rank rendezvous
        # BARRIER (and any cross-core launch skew) while phase-1 compute
        # runs, instead of paying it between phase 1 and the real AGs
        nc.gpsimd.collective_compute(
            "AllGather", mybir.AluOpType.bypass, replica_groups=groups,
            ins=[dummy_in.ap().opt()], outs=[dummy_out.ap().opt()])
        ones_f = constp.tile([P, 1], F32, tag="onesf")
        nc.any.memset(ones_f[:], 1.0)
        ones_col = constp.tile([P, 1], CD, tag="onesc")
        nc.vector.tensor_copy(ones_col[:], ones_f[:])
        onesr_f = constp.tile([1, P], F32, tag="onesrf")
        nc.any.memset(onesr_f[:], 1.0)
        ones_row = constp.tile([1, P], CD, tag="onesr")
        nc.vector.tensor_copy(ones_row[:], onesr_f[:])
        one_f11 = constp.tile([1, 1], F32, tag="onef11")
        nc.any.memset(one_f11[:], 1.0)
        one_1x1 = constp.tile([1, 1], CD, tag="one11")
        nc.vector.tensor_copy(one_1x1[:], one_f11[:])
        eps_t = constp.tile([1, 1], F32, tag="eps")
        nc.any.memset(eps_t[:], EPS)
        # touch Exp once now so the ACT table load happens during phase 1,
        # not at the first attention softmax
        warm_e = constp.tile([1, 1], F32, tag="warme")
        nc.scalar.activation(warm_e[:], eps_t[:], AFT.Exp, bias=0.0, scale=1.0)

        _wp3cm = tc.tile_pool(name="p3w", bufs=1)
        wp3 = _wp3cm.__enter__()
        wo_sb = [None] * (DIM // P)

        # persistent phase-2 tensors; zero-fills go first on the gpsimd queue
        _qp2cm = tc.tile_pool(name="p2q", bufs=1)
        qp2 = _qp2cm.__enter__()
        _kp2cm = tc.tile_pool(name="p2k", bufs=1)
        kp2 = _kp2cm.__enter__()
        _vp2cm = tc.tile_pool(name="p2v", bufs=T // P)
        vp2 = _vp2cm.__enter__()
        q_n = [qp2.tile([P, T], CD, tag=f"qn{h}", name=f"qn{h}") for h in range(HPC)]
        q_rope = [qp2.tile([P, T], CD, tag=f"qrope{h}", name=f"qrope{h}")
                  for h in range(HPC)]
        k_n = [kp2.tile([P, T], CD, tag=f"kn{h}", name=f"kn{h}") for h in range(HPC)]
        k_pe = kp2.tile([2 * DR, T], CD, tag="kpe")
        v_tok = [vp2.tile([P, HPC * DV], CD, tag="vtok", name=f"vtok{i}")
                 for i in range(T // P)]
        nc.any.memset(k_pe[DR:2 * DR, :], 0.0)
        for h in range(HPC):
            nc.any.memset(q_rope[h][DR:2 * DR, :], 0.0)

        # fused q weight loads NOW (t~0) so the fused-q chunks can start the
        # instant phase 1 finishes (SBUF for this is freed below by keeping
        # the phase-1 square tiles in a ring instead of 16 persistent tiles)
        _wqecm = tc.tile_pool(name="p2wqe", bufs=1)
        wqep = _wqecm.__enter__()
        wqe_sb = []
        for m in range(3):
            w = wqep.tile([P, DIM], CD, tag=f"wqe{m}", name=f"wqe{m}")
            nc.sync.dma_start(w[:], wqeT[m, :, :])
            wqe_sb.append(w)

        # ---------------- phase 1 ------------------------------------------
        with tc.tile_pool(name="p1x", bufs=1) as xp, \
             tc.tile_pool(name="p1w", bufs=3) as wp1, \
             tc.tile_pool(name="p1ps", bufs=3, space="PSUM") as ps1, \
             tc.tile_pool(name="p1ssps", bufs=2, space="PSUM") as ssps1, \
             tc.tile_pool(name="p1sq", bufs=4) as sqp, \
             tc.tile_pool(name="p1misc", bufs=2) as mp1, \
             tc.tile_pool(name="p1out", bufs=3) as op1:

            # first kv weight tile goes out ahead of the x tiles so the
            # first matmul isn't stuck behind 2MB of x DMA
            wkv0 = wp1.tile([P, DIM], CD, tag="w1")
            nc.sync.dma_start(wkv0[:], wkvaT[0, :, :])

            xts = []
            for kb in range(DIM // P):
                xt = xp.tile([P, TPC], CD, tag=f"x{kb}")
                nc.sync.dma_start(xt[:], xT[kb * P:(kb + 1) * P, :])
                xts.append(xt)

            cosc_sb = mp1.tile([P, TPC], CD, tag="cosc")
            nc.sync.dma_start(cosc_sb[:], cosc[:, :])
            sinc_sb = mp1.tile([P, TPC], CD, tag="sinc")
            nc.sync.dma_start(sinc_sb[:], sinc[:, :])

            def lora_proj(w_param, nm, out_cb, w0=None):
                for m in range(nm):
                    if m == 0 and w0 is not None:
                        wt = w0
                    else:
                        wt = wp1.tile([P, DIM], CD, tag="w1")
                        nc.sync.dma_start(wt[:], w_param[m, :, :])
                    ps = ps1.tile([P, TPC], F32, tag="ps1")
                    for kb in range(DIM // P):
                        nc.tensor.matmul(ps[:], wt[:, kb * P:(kb + 1) * P], xts[kb][:],
                                         start=(kb == 0), stop=(kb == DIM // P - 1))
                    out_cb(m, ps)

            def invrms_row(ss, nfeat, dst, dst_row):
                """accumulated sumsq -> 1/rms row (CD) -> DMA to dst[dst_row]."""
                srt = mp1.tile([1, TPC], F32, tag="srt")
                nc.scalar.activation(srt[:], ss[:], AFT.Sqrt,
                                     bias=eps_t[:], scale=1.0 / nfeat)
                inv_f = mp1.tile([1, TPC], F32, tag="invf")
                nc.vector.reciprocal_approx_fast(inv_f[:], srt[:])
                inv = mp1.tile([1, TPC], CD, tag="inv")
                nc.vector.tensor_copy(inv[:], inv_f[:])
                nc.scalar.dma_start(dst[dst_row:dst_row + 1, :], inv[:])

            # sumsq accumulated INLINE (ss matmul per m-tile) so the square
            # tiles live in a 4-deep ring instead of 16 persistent tiles --
            # that SBUF headroom is what pays for the early wqe load above
            ss_kv = ssps1.tile([1, TPC], F32, tag="sskv")
            ss_q = ssps1.tile([1, TPC], F32, tag="ssq")

            # --- kv branch (payload shipped unnormalized + inv-rms row)
            def kv_cb(m, ps):
                if m < 4:
                    kv = op1.tile([P, TPC], CD, tag="pay")
                    nc.scalar.copy(kv[:], ps[:])
                    nc.scalar.dma_start(agkv_in[m * P:(m + 1) * P, :], kv[:])
                    sq = sqp.tile([P, TPC], CD, tag="sq")
                    nc.scalar.square(sq[:], ps[:])
                    nc.tensor.matmul(ss_kv[:], ones_col[:], sq[:],
                                     start=(m == 0), stop=(m == 3))
                else:
                    # rows 0:64 deinterleaved kpe [r;i], rows 64:128 [i;r]
                    t_a = op1.tile([DR, TPC], CD, tag="ropea")
                    nc.vector.tensor_mul(t_a[:], ps[0:DR, :], cosc_sb[0:DR, :])
                    t_b = op1.tile([DR, TPC], CD, tag="ropeb")
                    nc.vector.tensor_mul(t_b[:], ps[DR:2 * DR, :], sinc_sb[0:DR, :])
                    kpe = op1.tile([DR, TPC], CD, tag="ropeo")
                    nc.vector.tensor_add(kpe[:], t_a[:], t_b[:])
                    nc.scalar.dma_start(agkv_in[KVL:KVL + DR, :], kpe[:])

            lora_proj(wkvaT, KVE // P, kv_cb, w0=wkv0)
            invrms_row(ss_kv, KVL, agkv_in, KVL + DR)

            # --- q branch: only the rms statistic is needed token-parallel;
            # it rides the kv AllGather as one extra row. The q heads are
            # produced by the fused W_eff = wq_a.T @ wq_b.T weight over ALL
            # tokens in phase 2 (no q AllGather at all).
            def q_cb(m, ps):
                sq = sqp.tile([P, TPC], CD, tag="sq")
                nc.scalar.square(sq[:], ps[:])
                nc.tensor.matmul(ss_q[:], ones_col[:], sq[:],
                                 start=(m == 0), stop=(m == QL // P - 1))

            lora_proj(wqaT, QL // P, q_cb)
            invrms_row(ss_q, QL, agkv_in, KVL + DR + 1)

            nc.gpsimd.collective_compute(
                "AllGather", mybir.AluOpType.bypass, replica_groups=groups,
                ins=[agkv_in.ap().opt()], outs=[agkv_out.ap().opt()])

        # ---------------- phase 2 ------------------------------------------
        if True:
            with tc.tile_pool(name="p2cs", bufs=2) as csp, \
                 tc.tile_pool(name="p2w", bufs=1) as wp2, \
                 tc.tile_pool(name="p2actq", bufs=12) as actp, \
                 tc.tile_pool(name="p2actkv", bufs=6) as actkvp, \
                 tc.tile_pool(name="p2misc", bufs=3) as mp2, \
                 tc.tile_pool(name="p2nrm", bufs=3) as nrmp:
                perm_sb = mp2.tile([P, P], CD, tag="perm")
                nc.sync.dma_start(perm_sb[:], perm[:, :])
                wkvk_sb = []
                for m in range(2):
                    w = wp2.tile([P, KVL], CD, tag=f"wkvk{m}", name=f"wkvk{m}")
                    nc.sync.dma_start(w[:], wkvbTk[m, :, :])
                    wkvk_sb.append(w)
                wkvv_sb = []
                for kb in range(KVL // P):
                    w = wp2.tile([P, 2 * DV], CD, tag=f"wkvv{kb}", name=f"wkvv{kb}")
                    nc.sync.dma_start(w[:], wkvbTv[kb * P:(kb + 1) * P, :])
                    wkvv_sb.append(w)

                # --- fused q production: q heads for ALL tokens from local
                # x (replicated input) and the host-fused W_eff weight; only
                # the tiny inv-rms AG is a cross-core dependency, so chunks
                # 0..2 fill the tensor-idle window while AG_kv is in flight
                def q_chunk(qc, psq, psqb):
                    c0 = qc * CHUNK
                    iq = nrmp.tile([1, CHUNK], CD, tag="iq")
                    nc.sync.dma_start(
                        iq[:],
                        agkv_out[qc * KVG + KVL + DR + 1:
                                 qc * KVG + KVL + DR + 2, :])
                    bq_ps = psqb.tile([P, CHUNK], F32, tag="bq")
                    nc.tensor.matmul(bq_ps[:], ones_row[:], iq[:],
                                     start=True, stop=True)
                    bq = nrmp.tile([P, CHUNK], CD, tag="bqsb")
                    nc.scalar.copy(bq[:], bq_ps[:])
                    pss = [psq.tile([P, CHUNK], F32, tag=f"qps{m}", name=f"qps{m}")
                           for m in range(3)]
                    for kb in range(DIM // P):
                        at = actp.tile([P, CHUNK], CD, tag="actq")
                        nc.sync.dma_start(at[:], xTf[kb, :, c0:c0 + CHUNK])
                        for m in range(3):
                            nc.tensor.matmul(pss[m][:],
                                             wqe_sb[m][:, kb * P:(kb + 1) * P],
                                             at[:],
                                             start=(kb == 0),
                                             stop=(kb == DIM // P - 1))
                    for h in range(HPC):
                        nc.vector.tensor_mul(q_n[h][:, c0:c0 + CHUNK],
                                             pss[h][:], bq[:])
                    qr_raw = mp2.tile([P, CHUNK], CD, tag="qrraw")
                    nc.vector.tensor_mul(qr_raw[:], pss[2][:], bq[:])
                    ps_sw = psq.tile([P, CHUNK], F32, tag="qps0")
                    nc.tensor.matmul(ps_sw[:], perm_sb[:], qr_raw[:],
                                     start=True, stop=True)
                    pcol = c0 % S
                    cos_q = csp.tile([P, CHUNK], CD, tag="cosq")
                    nc.sync.dma_start(cos_q[:], cosb[:, pcol:pcol + CHUNK])
                    sin_q = csp.tile([P, CHUNK], CD, tag="sinq")
                    nc.sync.dma_start(sin_q[:], sinb[:, pcol:pcol + CHUNK])
                    t_a = mp2.tile([P, CHUNK], CD, tag="qra")
                    nc.vector.tensor_mul(t_a[:], qr_raw[:], cos_q[:])
                    t_b = mp2.tile([P, CHUNK], CD, tag="qrb")
                    nc.vector.tensor_mul(t_b[:], ps_sw[:], sin_q[:])
                    qr_fin = mp2.tile([P, CHUNK], CD, tag="qrfin")
                    nc.vector.tensor_add(qr_fin[:], t_a[:], t_b[:])
                    for h in range(HPC):
                        nc.sync.dma_start(q_rope[h][0:DR, c0:c0 + CHUNK],
                                          qr_fin[h * DR:(h + 1) * DR, :])

                qscope = tc.tile_pool(name="p2psq", bufs=2, space="PSUM")
                psq = qscope.__enter__()
                qbscope = tc.tile_pool(name="p2psqb", bufs=2, space="PSUM")
                psqb = qbscope.__enter__()
                for qc in range(3):
                    q_chunk(qc, psq, psqb)
                qbscope.__exit__(None, None, None)
                qscope.__exit__(None, None, None)

                # --- k/v production (gated only on AG_kv)
                kvscope = tc.tile_pool(name="p2psk", bufs=1, space="PSUM")
                psk = kvscope.__enter__()
                vscope = tc.tile_pool(name="p2psv", bufs=3, space="PSUM")
                psv = vscope.__enter__()
                bscope = tc.tile_pool(name="p2psb", bufs=1, space="PSUM")
                psb = bscope.__enter__()
                for qc in range(NCH):
                    c0 = qc * CHUNK
                    actkv_ts = []
                    for kb in range(KVL // P):
                        at = actkvp.tile([P, CHUNK], CD, tag="actkv")
                        nc.sync.dma_start(
                            at[:],
                            agkv_out[qc * KVG + kb * P: qc * KVG + (kb + 1) * P, :])
                        actkv_ts.append(at)
                    ikv = nrmp.tile([1, CHUNK], CD, tag="ikv")
                    nc.sync.dma_start(
                        ikv[:],
                        agkv_out[qc * KVG + KVL + DR: qc * KVG + KVL + DR + 1, :])
                    bkv_ps = psb.tile([P, CHUNK], F32, tag="bkv")
                    nc.tensor.matmul(bkv_ps[:], ones_row[:], ikv[:],
                                     start=True, stop=True)
                    bkv = nrmp.tile([P, CHUNK], CD, tag="bkvsb")
                    nc.scalar.copy(bkv[:], bkv_ps[:])
                    kps = [psk.tile([P, CHUNK], F32, tag=f"kps{m}", name=f"kps{m}")
                           for m in range(2)]
                    for kb in range(KVL // P):
                        for m in range(2):
                            nc.tensor.matmul(kps[m][:],
                                             wkvk_sb[m][:, kb * P:(kb + 1) * P],
                                             actkv_ts[kb][:],
                                             start=(kb == 0), stop=(kb == KVL // P - 1))
                    for h in range(HPC):
                        nc.vector.tensor_mul(k_n[h][:, c0:c0 + CHUNK], kps[h][:], bkv[:])
                    nc.sync.dma_start(
                        k_pe[0:DR, c0:c0 + CHUNK],
                        agkv_out[qc * KVG + KVL: qc * KVG + KVL + DR, :])
                    for ktc in range(CHUNK // P):
                        # inv_kv as a [128,1] column for this kt tile (K=1 matmul)
                        col_ps = psb.tile([P, 1], F32, tag="colp")
                        nc.tensor.matmul(col_ps[:],
                                         ikv[0:1, ktc * P:(ktc + 1) * P],
                                         one_1x1[:], start=True, stop=True)
                        vcol = nrmp.tile([P, 1], F32, tag="vcol")
                        nc.scalar.copy(vcol[:], col_ps[:])
                        vps = psv.tile([P, HPC * DV], F32, tag="vps")
                        for kb in range(KVL // P):
                            nc.tensor.matmul(vps[:],
                                             actkv_ts[kb][:, ktc * P:(ktc + 1) * P],
                                             wkvv_sb[kb][:],
                                             start=(kb == 0), stop=(kb == KVL // P - 1))
                        nc.scalar.activation(v_tok[qc * (CHUNK // P) + ktc][:], vps[:],
                                             AFT.Copy, bias=0.0, scale=vcol[:])
                bscope.__exit__(None, None, None)
                vscope.__exit__(None, None, None)
                kvscope.__exit__(None, None, None)

                # --- remaining fused-q chunks
                qscope = tc.tile_pool(name="p2psq2", bufs=2, space="PSUM")
                psq = qscope.__enter__()
                qbscope = tc.tile_pool(name="p2psqb2", bufs=2, space="PSUM")
                psqb = qbscope.__enter__()
                for qc in range(3, NCH):
                    q_chunk(qc, psq, psqb)
                qbscope.__exit__(None, None, None)
                qscope.__exit__(None, None, None)
            _wqecm.__exit__(None, None, None)

            # prefetch half the phase-3 wo weights while attention runs
            for m in range(DIM // P // 2):
                wt = wp3.tile([P, DIM], CD, tag=f"wo{m}", name=f"wo{m}")
                nc.sync.dma_start(wt[:], woT[m, :, :])
                wo_sb[m] = wt

            # ----- causal attention (S^T formulation), head-outer for A2A overlap
            # kt-pairs share one [128,1024] PSUM score tile so exp runs as a
            # single wide ACTIVATE (halves the per-instruction overhead); the
            # causal diagonal is masked by a DVE multiply with precomputed 0/1
            # masks (keeps gpsimd off the critical path); outputs are
            # normalized here (den broadcast + reciprocal) so the A2A ships
            # only DV rows and phase 3 is a pure matmul
            with tc.tile_pool(name="apt", bufs=4) as ptp, \
                 tc.tile_pool(name="amask", bufs=1) as mkp, \
                 tc.tile_pool(name="aout", bufs=2, space="PSUM") as outp, \
                 tc.tile_pool(name="aden", bufs=1, space="PSUM") as denp, \
                 tc.tile_pool(name="abc", bufs=1, space="PSUM") as bcp, \
                 tc.tile_pool(name="ast", bufs=2, space="PSUM") as stp, \
                 tc.tile_pool(name="afin", bufs=3) as finp:
                dmask_sb = mkp.tile([P, 4 * CHUNK], CD, tag="dmask")
                nc.sync.dma_start(dmask_sb[:], dmaskP[:, :])
                for h in range(HPC):
                    for b in range(B):
                        for qcl in range(S // CHUNK):
                            qg = b * (S // CHUNK) + qcl
                            q0 = qg * CHUNK
                            nkt = (CHUNK // P) * (qcl + 1)
                            npair = nkt // 2
                            out_ps = outp.tile([P, CHUNK], F32, tag="out")
                            den_ps = denp.tile([1, CHUNK], F32, tag="den")

                            def qk_pair(pi):
                                st_ps = stp.tile([P, 2 * CHUNK], F32, tag="st",
                                                 name=f"st{pi}")
                                for s_ in range(2):
                                    kt = 2 * pi + s_
                                    kcol = b * S + kt * P
                                    sl = st_ps[:, s_ * CHUNK:(s_ + 1) * CHUNK]
                                    nc.tensor.matmul(sl,
                                                     k_n[h][:, kcol:kcol + P],
                                                     q_n[h][:, q0:q0 + CHUNK],
                                                     start=True, stop=False)
                                for s_ in range(2):
                                    kt = 2 * pi + s_
                                    kcol = b * S + kt * P
                                    nc.tensor.matmul(
                                        st_ps[:, s_ * CHUNK:(s_ + 1) * CHUNK],
                                        k_pe[:, kcol:kcol + P],
                                        q_rope[h][:, q0:q0 + CHUNK],
                                        start=False, stop=True)
                                pt = ptp.tile([P, 2 * CHUNK], CD, tag="pt",
                                              name=f"pt{pi}")
                                nc.scalar.activation(pt[:], st_ps[:], AFT.Exp,
                                                     bias=0.0, scale=SCALE)
                                halves = []
                                for s_ in range(2):
                                    kt = 2 * pi + s_
                                    r = kt - (CHUNK // P) * qcl
                                    if r >= 0:  # diagonal block: mask j >= p+128r
                                        ptm = ptp.tile([P, CHUNK], CD, tag="ptm",
                                                       name=f"ptm{pi}_{s_}")
                                        nc.vector.tensor_mul(
                                            ptm[:],
                                            pt[:, s_ * CHUNK:(s_ + 1) * CHUNK],
                                            dmask_sb[:, r * CHUNK:(r + 1) * CHUNK])
                                        halves.append(ptm[:])
                                    else:
                                        halves.append(
                                            pt[:, s_ * CHUNK:(s_ + 1) * CHUNK])
                                return halves

                            pend = {}

                            def av_pair(pi, halves):
                                for s_ in range(2):
                                    kt = 2 * pi + s_
                                    nc.tensor.matmul(
                                        out_ps[:],
                                        v_tok[(b * S // P) + kt][:,
                                                                 h * DV:(h + 1) * DV],
                                        halves[s_],
                                        start=(kt == 0), stop=(kt == nkt - 1))
                                pd = ptp.tile([P, CHUNK], CD, tag="pden",
                                              name=f"pden{pi}")
                                nc.vector.tensor_add(pd[:], halves[0],
                                                     halves[1])
                                if pi % 2 == 0:
                                    pend[0] = pd
                                else:
                                    pd4 = ptp.tile([P, CHUNK], CD, tag="pden4",
                                                   name=f"pden4_{pi}")
                                    nc.vector.tensor_add(pd4[:], pend[0][:], pd[:])
                                    nc.tensor.matmul(den_ps[:], ones_col[:], pd4[:],
                                                     start=(pi == 1),
                                                     stop=(pi == npair - 1))

                            prev = None
                            for pi in range(npair):
                                cur = qk_pair(pi)
                                if prev is not None:
                                    av_pair(pi - 1, prev)
                                prev = cur
                            av_pair(npair - 1, prev)
                            # normalize here: broadcast den to 128 rows,
                            # reciprocal, scale the output as it leaves PSUM
                            den_c = finp.tile([1, CHUNK], CD, tag="denc")
                            nc.scalar.copy(den_c[:], den_ps[:])
                            bc_ps = bcp.tile([P, CHUNK], F32, tag="bc")
                            nc.tensor.matmul(bc_ps[:], ones_row[:], den_c[:],
                                             start=True, stop=True)
                            binv = finp.tile([P, CHUNK], F32, tag="binv")
                            nc.vector.reciprocal_approx_fast(binv[:], bc_ps[:])
                            attn = finp.tile([P, CHUNK], CD, tag="attn")
                            nc.vector.tensor_mul(attn[:], out_ps[:], binv[:])
                            nc.sync.dma_start(
                                a2a_in[h][qg * DV:(qg + 1) * DV, :], attn[:])
                    nc.gpsimd.collective_compute(
                        "AllToAll", mybir.AluOpType.bypass, replica_groups=groups,
                        ins=[a2a_in[h].ap().opt()], outs=[a2a_out[h].ap().opt()])

        _vp2cm.__exit__(None, None, None)
        _kp2cm.__exit__(None, None, None)
        _qp2cm.__exit__(None, None, None)

        # ---------------- phase 3 ------------------------------------------
        with tc.tile_pool(name="p3r", bufs=1) as rp3, \
             tc.tile_pool(name="p3ps", bufs=2, space="PSUM") as ps3, \
             tc.tile_pool(name="p3o", bufs=3) as op3:
            # heads arriving via the first A2A load first; the woT weight
            # loads sit between the two gated groups so they prefetch
            # before the second A2A lands
            rhs_t = [None] * H
            order = [g for g in range(H) if g % HPC == 0]
            order2 = [g for g in range(H) if g % HPC != 0]

            def load_head(g):
                blk = (g // HPC) * DV
                rt = rp3.tile([P, CHUNK], CD, tag=f"r{g}", name=f"r{g}")
                nc.sync.dma_start(rt[:], a2a_out[g % HPC][blk:blk + DV, :])
                rhs_t[g] = rt

            for m in range(DIM // P // 2, DIM // P):
                wt = wp3.tile([P, DIM], CD, tag=f"wo{m}", name=f"wo{m}")
                nc.sync.dma_start(wt[:], woT[m, :, :])
                wo_sb[m] = wt
            for g in order:
                load_head(g)
            for g in order2:
                load_head(g)

            partials = []
            for m in range(DIM // P):
                ps = ps3.tile([P, CHUNK], F32, tag="ps3")
                for i, g in enumerate(order):
                    nc.tensor.matmul(ps[:], wo_sb[m][:, g * P:(g + 1) * P], rhs_t[g][:],
                                     start=(i == 0), stop=(i == len(order) - 1))
                pa = rp3.tile([P, CHUNK], F32, tag=f"pa{m}", name=f"pa{m}")
                nc.scalar.copy(pa[:], ps[:])
                partials.append(pa)
            for m in range(DIM // P):
                ps = ps3.tile([P, CHUNK], F32, tag="ps3")
                for i, g in enumerate(order2):
                    nc.tensor.matmul(ps[:], wo_sb[m][:, g * P:(g + 1) * P], rhs_t[g][:],
                                     start=(i == 0), stop=(i == len(order2) - 1))
                ot = op3.tile([P, CHUNK], CD, tag="ot")
                nc.vector.tensor_add(ot[:], ps[:], partials[m][:])
                nc.sync.dma_start(outT[m * P:(m + 1) * P, :], ot[:])
        _wp3cm.__exit__(None, None, None)

    nc.compile()
    return nc


def _tile_kxm(w, nk, nm):
    """(nk*128, nm*128) -> (nm, 128, nk*128): [m][p][kt*128+j] = w[kt*128+p, m*128+j]."""
    return np.ascontiguousarray(
        w.reshape(nk, P, nm, P).transpose(2, 1, 0, 3).reshape(nm, P, nk * P))


_CACHE = {}


def _prep(inputs):
    x = np.asarray(inputs["x"], np.float32)
    fc = np.asarray(inputs["freqs_cos"], np.float32)
    fs = np.asarray(inputs["freqs_sin"], np.float32)
    wq_a = np.asarray(inputs["wq_a"], np.float32)
    q_norm_w = np.asarray(inputs["q_norm_w"], np.float32)
    wq_b = np.asarray(inputs["wq_b"], np.float32)
    wkv_a = np.asarray(inputs["wkv_a"], np.float32)
    kv_norm_w = np.asarray(inputs["kv_norm_w"], np.float32)
    wkv_b = np.asarray(inputs["wkv_b"], np.float32)
    wo = np.asarray(inputs["wo"], np.float32)

    x_flat = x.reshape(T, DIM)
    xTf_t = np.ascontiguousarray(x_flat.T).reshape(DIM // P, P, T).astype(BF)

    wqaT_t = _tile_kxm(wq_a.T, DIM // P, QL // P)

    at = wkv_a.T                                     # (DIM, 576)
    Rw = at[:, KVL::2]
    Iw = at[:, KVL + 1::2]
    wkvaT_t = _tile_kxm(np.concatenate([at[:, :KVL], Rw, Iw, Iw, Rw], axis=1),
                        DIM // P, KVE // P)

    wqb_sT = (wq_b * q_norm_w[None, :]).T            # (QL, H*192)
    wkvb_sT = (wkv_b * kv_norm_w[None, :]).T         # (KVL, H*256)

    woT_t = _tile_kxm(wo.T, DIM // P, DIM // P)

    cT, sT = fc.T, fs.T
    cosbM = np.concatenate([cT, cT, cT, cT], axis=0)
    sinbM = np.concatenate([-sT, sT, -sT, sT], axis=0)
    permM = np.zeros((P, P), np.float32)
    permM[np.arange(P) ^ 32, np.arange(P)] = 1.0

    # causal diagonal masks: mask_r[p, j] = (j >= p + 128*r) for the 4
    # possible k-tile offsets within a 512-token q chunk
    jj = np.arange(CHUNK)[None, :]
    pp = np.arange(P)[:, None]
    dmaskM = np.concatenate(
        [(jj >= pp + P * r).astype(np.float32) for r in range(4)], axis=1)

    in_maps = []
    for c in range(NC):
        h0, h1 = 2 * c, 2 * c + 1
        qb = [wqb_sT[:, h * 192: h * 192 + DN] for h in (h0, h1)]
        for h in (h0, h1):
            rope = wqb_sT[:, h * 192 + DN:(h + 1) * 192]
            qb.append(rope[:, 0::2])
            qb.append(rope[:, 1::2])
        # fold wq_a through: q^h = inv_rms ⊙ (x @ (wq_a.T @ wqb_sT^h))
        wqe_c = wq_a.T @ np.concatenate(qb, axis=1)      # (DIM, 384)
        wqeT_ct = _tile_kxm(wqe_c, DIM // P, 3)

        kn = [wkvb_sT[:, h * 256: h * 256 + DN] for h in (h0, h1)]
        vv = [wkvb_sT[:, h * 256 + DN: (h + 1) * 256] for h in (h0, h1)]
        wkvbTk_c = _tile_kxm(np.concatenate(kn, axis=1), KVL // P, 2)
        wkvbTv_c = np.ascontiguousarray(np.concatenate(vv, axis=1))

        pos0 = (c * TPC) % S
        in_maps.append({
            "xT": np.ascontiguousarray(x_flat[c * TPC:(c + 1) * TPC].T).astype(BF),
            "wqaT": wqaT_t.astype(BF), "wkvaT": wkvaT_t.astype(BF),
            "wqeT": wqeT_ct.astype(BF), "xTf": xTf_t,
            "wkvbTk": wkvbTk_c.astype(BF),
            "wkvbTv": wkvbTv_c.astype(BF),
            "woT": woT_t.astype(BF), "cosb": cosbM.astype(BF),
            "sinb": sinbM.astype(BF), "perm": permM.astype(BF),
            "dmaskP": dmaskM.astype(BF),
            "cosc": np.ascontiguousarray(cosbM[:, pos0:pos0 + TPC]).astype(BF),
            "sinc": np.ascontiguousarray(sinbM[:, pos0:pos0 + TPC]).astype(BF),
        })
    return in_maps


def kernel(**inputs):
    in_maps = _prep(inputs)
    if "nc" not in _CACHE:
        _CACHE["nc"] = build_nc()
    r = run_bass_kernel_spmd(_CACHE["nc"], in_maps, list(range(NC)))
    out_flat = np.empty((T, DIM), np.float32)
    for c in range(NC):
        out_flat[c * TPC:(c + 1) * TPC] = r.results[c]["outT"].T.astype(np.float32)
    return out_flat.reshape(B, S, DIM)

